# revision 12
# baseline (speedup 1.0000x reference)
"""DigitCaps (CapsNet dynamic routing) kernel for 8 Trainium2 NeuronCores.

Reference math:
  u_hat[b,r,c,o] = sum_i W[r,c,o,i] * x[b,r,i]
  b_ij = 0;  3 routing iterations:
     c = softmax_r(b);  s[b,c,o] = sum_r c[r,c] u_hat[b,r,c,o];
     v = squash(s);     b += mean_b(sum_o u_hat[b,r,c,o] v[b,c,o])
  returns v[..., None]  (256, 10, 16, 1)

Strategy: data-parallel over batch (32 per core), W replicated.  u_hat
(189 MB) is NEVER materialized — the routing coefficients are folded into
the weights so every pass is a dense matmul over the contraction dim
K=(r,i)=9216:
    s-matmul:  s[b,(o,c)]   = sum_K  XT[K,b] * (c-scaled Wg)[K,(o,c)]
    G-matmul:  G[K,(o,c)]   = sum_b  xn[b,K] * (v[b,(o,c)]/B)
    agreement: P = Wg .* G; an accumulating indicator matmul (COMP)
               reduces i across partitions and lands the per-core partial
               abar[r,c] in pure-r partition layout; abar is AllReduced
               across the 8 cores (batch-shared b_ij update).
All matmuls run in bf16; softmax and squash run in fp32.

Perf notes for this runtime (axon-tunneled cores): per-call cost is
dominated by STATIC NEFF instruction count (~60us/instruction host-side
processing per execute) plus ~6ms per collective, and grows with per-core
matmul/copy volume; executed static-AP instructions are otherwise cheap.
So the big tile loops run as For_i hardware loops (loop COUNT matters —
each loop carries ~80 instructions of back-edge machinery): the G matmul
and the agreement COMP matmul are fused into one loop through a fixed
SBUF staging tile, the matmul stationary operand (which needs a static
offset) is staged via vector copies with dynamic source offsets, and the
host pre-arranges wg/xt partition-major so input DMAs are contiguous.
"""
import sys
if '/opt/trn_rl_repo' not in sys.path:
    sys.path.insert(0, '/opt/trn_rl_repo')
import numpy as np
import ml_dtypes

import concourse.bass as bass
import concourse.bacc as bacc
import concourse.mybir as mybir
import concourse.tile as tile
from concourse import bass_utils
from concourse.bass import ts

BF16 = mybir.dt.bfloat16
F32 = mybir.dt.float32

B, R, C, O, I = 256, 1152, 10, 16, 8
NCORES = 8
BL = B // NCORES          # 32 local batch
RT = 9                    # r tiles of 128
NT = 72                   # (r,i) tiles of 128
CO = C * O                # 160, free order (o,c): idx = o*C + c
NITER = 3

_CACHE = {}


def _build(n_cores=NCORES, reps=1):
    nc = bacc.Bacc("TRN2", target_bir_lowering=False, debug=False,
                   num_devices=n_cores)
    # wg/xt are pre-arranged by the host into the exact SBUF layout
    # (partition-major), so their DMAs are fully contiguous.
    wg_d = nc.dram_tensor("wg", [128, NT * CO], BF16, kind="ExternalInput")
    xt_d = nc.dram_tensor("xt", [128, NT * BL], BF16, kind="ExternalInput")
    xn_d = nc.dram_tensor("xn", [BL, R * I], BF16, kind="ExternalInput")
    rep_d = nc.dram_tensor("rep", [128, 8 * 128], F32, kind="ExternalInput")
    cmp_d = nc.dram_tensor("cmp", [128, 8 * 128], BF16, kind="ExternalInput")
    out_d = nc.dram_tensor("out", [BL, CO], F32, kind="ExternalOutput")

    with tile.TileContext(nc) as tc:
        with (
            tc.tile_pool(name="big", bufs=1) as big,
            tc.tile_pool(name="small", bufs=1) as small,
            tc.tile_pool(name="sps", bufs=1, space="PSUM") as sps,
            tc.tile_pool(name="gps", bufs=2, space="PSUM") as gps,
            tc.tile_pool(name="aps", bufs=2, space="PSUM") as aps,
            tc.tile_pool(name="zps", bufs=1, space="PSUM") as zps_pool,
            tc.tile_pool(name="dram", bufs=4, space="DRAM") as dram,
        ):
            Wg = big.tile([128, NT * CO], BF16, tag="Wg")
            Wp = big.tile([128, NT * CO], BF16, tag="Wp")
            ptile = big.tile([128, CO], BF16, tag="ptile")
            XT = big.tile([128, NT * BL], BF16, tag="XT")
            XN = big.tile([BL, R * I], BF16, tag="XN")
            REP = big.tile([128, 8 * 128], F32, tag="REP")
            CMP = big.tile([128, 8 * 128], BF16, tag="CMP")
            crep = big.tile([128, NT * C], BF16, tag="crep")
            xstage = big.tile([128, BL], BF16, tag="xstage")
            nstage = big.tile([BL, 128], BF16, tag="nstage")

            b_sb = small.tile([128, RT * C], F32, tag="b")
            expb = small.tile([128, RT * C], F32, tag="expb")
            c_sb = small.tile([128, RT * C], F32, tag="c")
            abar = small.tile([128, RT * C], F32, tag="abar")
            arr = small.tile([128, RT * C], F32, tag="arr")
            ones128 = small.tile([128, 1], F32, tag="ones128")
            ones1 = small.tile([1, 128], F32, tag="ones1")
            zr = small.tile([1, C], F32, tag="zr")
            se = small.tile([BL, CO], F32, tag="se")
            neg = small.tile([BL, CO], F32, tag="neg")
            ab = small.tile([BL, CO], F32, tag="ab")
            sq = small.tile([BL, CO], F32, tag="sq")
            den = small.tile([BL, CO], F32, tag="den")
            rd = small.tile([BL, CO], F32, tag="rd")
            num = small.tile([BL, CO], F32, tag="num")
            vv = small.tile([BL, CO], F32, tag="v")
            vbf = small.tile([BL, CO], BF16, tag="vbf")

            for _rep in range(reps):
              nc.sync.dma_start(out=Wg[:, :], in_=wg_d[:, :])
              nc.sync.dma_start(out=XT[:, :], in_=xt_d[:, :])
              nc.sync.dma_start(out=XN[:, :], in_=xn_d[:, :])
              nc.sync.dma_start(out=REP[:, :], in_=rep_d[:, :])
              nc.sync.dma_start(out=CMP[:, :], in_=cmp_d[:, :])
              nc.vector.memset(ones128[:, :], 1.0)
              nc.vector.memset(ones1[:, :], 1.0)
              nc.vector.memset(b_sb[:, :], 0.0)

              expb3 = expb[:, :].rearrange("p (t c) -> p t c", c=C)
              c3 = c_sb[:, :].rearrange("p (t c) -> p t c", c=C)
              abar3 = abar[:, :].rearrange("p (t c) -> p t c", c=C)

              for k in range(NITER):
                  if k > 0:
                      # c = softmax_r(b)
                      nc.scalar.activation(expb[:, :], b_sb[:, :],
                                           mybir.ActivationFunctionType.Exp)
                      zp = zps_pool.tile([1, C], F32, tag="zp")
                      for t in range(RT):
                          nc.tensor.matmul(zp[:, :], ones128[:, :],
                                           expb3[:, t, :],
                                           start=(t == 0), stop=(t == RT - 1))
                      nc.vector.reciprocal(zr[:, :], zp[:, :])
                      zbc = zps_pool.tile([128, C], F32, tag="zbc")
                      nc.tensor.matmul(zbc[:, :], ones1[:, :], zr[:, :],
                                       start=True, stop=True)
                      zbc_b = zbc[:, :].unsqueeze(1).broadcast_to((128, RT, C))
                      nc.vector.tensor_tensor(c3, expb3, zbc_b,
                                              op=mybir.AluOpType.mult)
                      # replicate c over i (partition x8) via indicator
                      # matmuls; one For_i over the 9 r-tiles
                      with tc.For_i(0, RT) as iv:
                          cps = zps_pool.tile([128, 8 * C], F32, tag="cps")
                          for s in range(8):
                              nc.tensor.matmul(
                                  cps[:, s * C:(s + 1) * C],
                                  REP[:, s * 128:(s + 1) * 128],
                                  c_sb[:, ts(iv, C)], start=True, stop=True)
                          nc.vector.tensor_copy(out=crep[:, ts(iv, 8 * C)],
                                                in_=cps[:, :])
                      # W' = Wg * crep (broadcast over o)
                      wg4 = Wg[:, :].rearrange("p (t o c) -> p t o c", o=O, c=C)
                      wp4 = Wp[:, :].rearrange("p (t o c) -> p t o c", o=O, c=C)
                      cr4 = crep[:, :].rearrange(
                          "p (t c) -> p t c", c=C).unsqueeze(2).broadcast_to(
                          (128, NT, O, C))
                      nc.vector.tensor_tensor(wp4, wg4, cr4,
                                              op=mybir.AluOpType.mult)

                  # s matmul over the 72 (r,i) tiles: stationary XT tile is
                  # staged through xstage (static offset), rhs is dynamic
                  mov = Wg if k == 0 else Wp
                  s_ps = sps.tile([BL, CO], F32, tag="s")
                  nc.tensor.matmul(s_ps[:, :], XT[:, 0:BL], mov[:, 0:CO],
                                   start=True, stop=False)
                  with tc.For_i(1, NT - 1) as iv:
                      nc.vector.tensor_copy(out=xstage[:, :],
                                            in_=XT[:, ts(iv, BL)])
                      nc.tensor.matmul(s_ps[:, :], xstage[:, :],
                                       mov[:, ts(iv, CO)],
                                       start=False, stop=False)
                  nc.tensor.matmul(s_ps[:, :], XT[:, (NT - 1) * BL:NT * BL],
                                   mov[:, (NT - 1) * CO:NT * CO],
                                   start=False, stop=True)
                  # squash: v = s*|s| / (1+s^2)
                  nc.scalar.activation(se[:, :], s_ps[:, :],
                                       mybir.ActivationFunctionType.Copy,
                                       scale=(1.0 / R if k == 0 else 1.0))
                  nc.vector.tensor_scalar_mul(neg[:, :], se[:, :], -1.0)
                  nc.vector.tensor_max(ab[:, :], se[:, :], neg[:, :])
                  nc.vector.tensor_mul(sq[:, :], se[:, :], se[:, :])
                  nc.vector.tensor_scalar_add(den[:, :], sq[:, :], 1.0)
                  nc.vector.reciprocal(rd[:, :], den[:, :])
                  nc.vector.tensor_mul(num[:, :], se[:, :], ab[:, :])
                  nc.vector.tensor_mul(vv[:, :], num[:, :], rd[:, :])

                  if k == NITER - 1:
                      nc.sync.dma_start(out=out_d[:, :], in_=vv[:, :])
                      continue

                  nc.scalar.activation(vbf[:, :], vv[:, :],
                                       mybir.ActivationFunctionType.Copy,
                                       scale=1.0 / B)

                  # Merged G + agreement loop over the 9 r-tiles: for each of
                  # the 8 (r,i) subtiles, compute G tile (matmul over local
                  # batch), multiply by Wg into a fixed staging tile, and
                  # feed it straight into the accumulating COMP matmul that
                  # reduces i and lands results in pure-r partition layout.
                  with tc.For_i(0, RT) as iv:
                      a_ps = aps.tile([128, CO], F32, tag="a")
                      for s in range(8):
                          nc.vector.tensor_copy(
                              out=nstage[:, :],
                              in_=XN[:, ts(iv * 8 + s, 128)])
                          g_ps = gps.tile([128, CO], F32, tag="g")
                          nc.tensor.matmul(g_ps[:, :], nstage[:, :],
                                           vbf[:, :], start=True, stop=True)
                          nc.vector.tensor_tensor(
                              ptile[:, :], Wg[:, ts(iv * 8 + s, CO)],
                              g_ps[:, :], op=mybir.AluOpType.mult)
                          nc.tensor.matmul(a_ps[:, :],
                                           CMP[:, s * 128:(s + 1) * 128],
                                           ptile[:, :],
                                           start=(s == 0), stop=(s == 7))
                      a_v = a_ps[:, :].rearrange("p (o c) -> p c o", c=C)
                      nc.vector.tensor_reduce(abar[:, ts(iv, C)], a_v,
                                              axis=mybir.AxisListType.X,
                                              op=mybir.AluOpType.add)

                  # AllReduce of abar across the 8 cores
                  ar_in = dram.tile([R, C], F32, tag="arin")
                  ar_out = dram.tile([R, C], F32, tag="arout")
                  nc.sync.dma_start(
                      out=ar_in[:, :].rearrange("(t p) c -> p t c", p=128),
                      in_=abar3)
                  nc.gpsimd.collective_compute(
                      "AllReduce", mybir.AluOpType.add,
                      replica_groups=[list(range(n_cores))],
                      ins=[ar_in[:, :].opt()],
                      outs=[ar_out[:, :].opt()])
                  nc.sync.dma_start(
                      out=arr[:, :].rearrange("p (t c) -> p t c", c=C),
                      in_=ar_out[:, :].rearrange("(t p) c -> p t c", p=128))
                  nc.vector.tensor_add(b_sb[:, :], b_sb[:, :], arr[:, :])

    nc.compile()
    return nc


def _host_inputs(x, W):
    # wg[(r,i), (o,c)] -> partition-major [128, t, CO] (t = (r,i)//128)
    wg = np.ascontiguousarray(
        W.transpose(0, 3, 2, 1).reshape(R * I, CO)).astype(ml_dtypes.bfloat16)
    wg_pm = np.ascontiguousarray(
        wg.reshape(NT, 128, CO).transpose(1, 0, 2).reshape(128, NT * CO))
    m = np.arange(128)
    rep = np.zeros((128, 8 * 128), np.float32)
    cmp_ = np.zeros((128, 8 * 128), np.float32)
    for s in range(8):
        rep[:, s * 128:(s + 1) * 128] = (
            np.arange(128)[:, None] == (s * 16 + m[None, :] // 8))
        cmp_[:, s * 128:(s + 1) * 128] = (
            m[None, :] == (s * 16 + np.arange(128)[:, None] // 8))
    cmp_ = cmp_.astype(ml_dtypes.bfloat16)
    in_maps = []
    for cidx in range(NCORES):
        xc = x[cidx * BL:(cidx + 1) * BL]
        xt = np.ascontiguousarray(
            xc.transpose(1, 2, 0).reshape(R * I, BL)).astype(ml_dtypes.bfloat16)
        xt_pm = np.ascontiguousarray(
            xt.reshape(NT, 128, BL).transpose(1, 0, 2).reshape(128, NT * BL))
        xn = np.ascontiguousarray(
            xc.reshape(BL, R * I)).astype(ml_dtypes.bfloat16)
        in_maps.append({"wg": wg_pm, "xt": xt_pm, "xn": xn,
                        "rep": rep, "cmp": cmp_})
    return in_maps


def kernel(x, W):
    x = np.ascontiguousarray(np.asarray(x, dtype=np.float32))
    W = np.ascontiguousarray(np.asarray(W, dtype=np.float32))
    assert x.shape == (B, R, I) and W.shape == (R, C, O, I)
    if "nc" not in _CACHE:
        _CACHE["nc"] = _build()
    nc = _CACHE["nc"]
    in_maps = _host_inputs(x, W)
    res = bass_utils.run_bass_kernel_spmd(nc, in_maps,
                                          core_ids=list(range(NCORES)))
    vs = [r["out"].reshape(BL, O, C).transpose(0, 2, 1) for r in res.results]
    return np.concatenate(vs, axis=0)[..., None].astype(np.float32)


# revision 13
# speedup vs baseline: 1.1490x; 1.1490x over previous
"""DigitCaps (CapsNet dynamic routing) kernel for 8 Trainium2 NeuronCores.

Reference math:
  u_hat[b,r,c,o] = sum_i W[r,c,o,i] * x[b,r,i]
  b_ij = 0;  3 routing iterations:
     c = softmax_r(b);  s[b,c,o] = sum_r c[r,c] u_hat[b,r,c,o];
     v = squash(s);     b += mean_b(sum_o u_hat[b,r,c,o] v[b,c,o])
  returns v[..., None]  (256, 10, 16, 1)

Strategy: data-parallel over batch (32 per core), W replicated.  u_hat
(189 MB) is NEVER materialized — the routing coefficients are folded into
the weights so every pass is a dense matmul over the contraction dim
K=(r,i)=9216:
    s-matmul:  s[b,(o,c)]   = sum_K  XT[K,b] * (c-scaled Wg)[K,(o,c)]
    G-matmul:  G[K,(o,c)]   = sum_b  xn[b,K] * (v[b,(o,c)]/B)
    agreement: P = Wg .* G; an accumulating indicator matmul (COMP)
               reduces i across partitions and lands the per-core partial
               abar[r,c] in pure-r partition layout; abar is AllReduced
               across the 8 cores (batch-shared b_ij update).
All matmuls run in bf16; softmax and squash run in fp32.

Perf notes for this runtime (axon-tunneled cores): per-call cost is
dominated by STATIC NEFF instruction count (~60us/instruction host-side
processing per execute) plus ~6ms per collective, and grows with per-core
matmul/copy volume; executed static-AP instructions are otherwise cheap.
So the big tile loops run as For_i hardware loops (loop COUNT matters —
each loop carries ~80 instructions of back-edge machinery): the G matmul
and the agreement COMP matmul are fused into one loop through a fixed
SBUF staging tile, the matmul stationary operand (which needs a static
offset) is staged via vector copies with dynamic source offsets, and the
host pre-arranges wg/xt partition-major so input DMAs are contiguous.
"""
import sys
if '/opt/trn_rl_repo' not in sys.path:
    sys.path.insert(0, '/opt/trn_rl_repo')
import numpy as np
import ml_dtypes

import concourse.bass as bass
import concourse.bacc as bacc
import concourse.mybir as mybir
import concourse.tile as tile
from concourse import bass_utils
from concourse.bass import ts

BF16 = mybir.dt.bfloat16
F32 = mybir.dt.float32

B, R, C, O, I = 256, 1152, 10, 16, 8
NCORES = 8
BL = B // NCORES          # 32 local batch
RT = 9                    # r tiles of 128
NT = 72                   # (r,i) tiles of 128
CO = C * O                # 160, free order (o,c): idx = o*C + c
NITER = 3

_CACHE = {}


def _build(n_cores=NCORES, reps=1):
    nc = bacc.Bacc("TRN2", target_bir_lowering=False, debug=False,
                   num_devices=n_cores)
    # wg/xt are pre-arranged by the host into the exact SBUF layout
    # (partition-major), so their DMAs are fully contiguous.
    wg_d = nc.dram_tensor("wg", [128, NT * CO], BF16, kind="ExternalInput")
    xt_d = nc.dram_tensor("xt", [128, NT * BL], BF16, kind="ExternalInput")
    xn_d = nc.dram_tensor("xn", [BL, R * I], BF16, kind="ExternalInput")
    rep_d = nc.dram_tensor("rep", [128, 8 * 128], F32, kind="ExternalInput")
    cmp_d = nc.dram_tensor("cmp", [128, 8 * 128], BF16, kind="ExternalInput")
    out_d = nc.dram_tensor("out", [BL, CO], F32, kind="ExternalOutput")

    with tile.TileContext(nc) as tc:
        with (
            tc.tile_pool(name="big", bufs=1) as big,
            tc.tile_pool(name="small", bufs=1) as small,
            tc.tile_pool(name="sps", bufs=1, space="PSUM") as sps,
            tc.tile_pool(name="gps", bufs=2, space="PSUM") as gps,
            tc.tile_pool(name="aps", bufs=2, space="PSUM") as aps,
            tc.tile_pool(name="zps", bufs=1, space="PSUM") as zps_pool,
            tc.tile_pool(name="dram", bufs=4, space="DRAM") as dram,
        ):
            Wg = big.tile([128, NT * CO], BF16, tag="Wg")
            Wp = big.tile([128, NT * CO], BF16, tag="Wp")
            ptile = big.tile([128, CO], BF16, tag="ptile")
            XT = big.tile([128, NT * BL], BF16, tag="XT")
            XN = big.tile([BL, R * I], BF16, tag="XN")
            REP = big.tile([128, 8 * 128], F32, tag="REP")
            CMP = big.tile([128, 8 * 128], BF16, tag="CMP")
            crep = big.tile([128, NT * C], BF16, tag="crep")
            xstage = big.tile([128, BL], BF16, tag="xstage")
            nstage = big.tile([BL, 128], BF16, tag="nstage")

            b_sb = small.tile([128, RT * C], F32, tag="b")
            expb = small.tile([128, RT * C], F32, tag="expb")
            c_sb = small.tile([128, RT * C], F32, tag="c")
            abar = small.tile([128, RT * C], F32, tag="abar")
            arr = small.tile([128, RT * C], F32, tag="arr")
            ones128 = small.tile([128, 1], F32, tag="ones128")
            ones1 = small.tile([1, 128], F32, tag="ones1")
            zr = small.tile([1, C], F32, tag="zr")
            se = small.tile([BL, CO], F32, tag="se")
            neg = small.tile([BL, CO], F32, tag="neg")
            ab = small.tile([BL, CO], F32, tag="ab")
            sq = small.tile([BL, CO], F32, tag="sq")
            den = small.tile([BL, CO], F32, tag="den")
            rd = small.tile([BL, CO], F32, tag="rd")
            num = small.tile([BL, CO], F32, tag="num")
            vv = small.tile([BL, CO], F32, tag="v")
            vbf = small.tile([BL, CO], BF16, tag="vbf")

            for _rep in range(reps):
              nc.sync.dma_start(out=Wg[:, :], in_=wg_d[:, :])
              nc.sync.dma_start(out=XT[:, :], in_=xt_d[:, :])
              nc.sync.dma_start(out=XN[:, :], in_=xn_d[:, :])
              nc.sync.dma_start(out=REP[:, :], in_=rep_d[:, :])
              nc.sync.dma_start(out=CMP[:, :], in_=cmp_d[:, :])
              nc.vector.memset(ones128[:, :], 1.0)
              nc.vector.memset(ones1[:, :], 1.0)
              nc.vector.memset(b_sb[:, :], 0.0)

              expb3 = expb[:, :].rearrange("p (t c) -> p t c", c=C)
              c3 = c_sb[:, :].rearrange("p (t c) -> p t c", c=C)
              abar3 = abar[:, :].rearrange("p (t c) -> p t c", c=C)

              for k in range(NITER):
                  if k > 0:
                      # c = softmax_r(b)
                      nc.scalar.activation(expb[:, :], b_sb[:, :],
                                           mybir.ActivationFunctionType.Exp)
                      # partition sum of exp(b): collapse the 9 r-tiles with
                      # one strided free-axis reduce, then a single
                      # ones-matmul over the 128 partitions
                      ebs = small.tile([128, C], F32, tag="ebs")
                      e_v = expb[:, :].rearrange("p (t c) -> p c t", c=C)
                      nc.vector.tensor_reduce(ebs[:, :], e_v,
                                              axis=mybir.AxisListType.X,
                                              op=mybir.AluOpType.add)
                      zp = zps_pool.tile([1, C], F32, tag="zp")
                      nc.tensor.matmul(zp[:, :], ones128[:, :], ebs[:, :],
                                       start=True, stop=True)
                      nc.vector.reciprocal(zr[:, :], zp[:, :])
                      zbc = zps_pool.tile([128, C], F32, tag="zbc")
                      nc.tensor.matmul(zbc[:, :], ones1[:, :], zr[:, :],
                                       start=True, stop=True)
                      zbc_b = zbc[:, :].unsqueeze(1).broadcast_to((128, RT, C))
                      nc.vector.tensor_tensor(c3, expb3, zbc_b,
                                              op=mybir.AluOpType.mult)
                      # replicate c over i (partition x8) via indicator
                      # matmuls; one For_i over the 9 r-tiles
                      with tc.For_i(0, RT) as iv:
                          cps = zps_pool.tile([128, 8 * C], F32, tag="cps")
                          for s in range(8):
                              nc.tensor.matmul(
                                  cps[:, s * C:(s + 1) * C],
                                  REP[:, s * 128:(s + 1) * 128],
                                  c_sb[:, ts(iv, C)], start=True, stop=True)
                          nc.vector.tensor_copy(out=crep[:, ts(iv, 8 * C)],
                                                in_=cps[:, :])
                      # W' = Wg * crep (broadcast over o)
                      wg4 = Wg[:, :].rearrange("p (t o c) -> p t o c", o=O, c=C)
                      wp4 = Wp[:, :].rearrange("p (t o c) -> p t o c", o=O, c=C)
                      cr4 = crep[:, :].rearrange(
                          "p (t c) -> p t c", c=C).unsqueeze(2).broadcast_to(
                          (128, NT, O, C))
                      nc.vector.tensor_tensor(wp4, wg4, cr4,
                                              op=mybir.AluOpType.mult)

                  # s matmul over the 72 (r,i) tiles: stationary XT tile is
                  # staged through xstage (static offset), rhs is dynamic
                  mov = Wg if k == 0 else Wp
                  s_ps = sps.tile([BL, CO], F32, tag="s")
                  nc.tensor.matmul(s_ps[:, :], XT[:, 0:BL], mov[:, 0:CO],
                                   start=True, stop=False)
                  with tc.For_i(1, NT - 1) as iv:
                      nc.vector.tensor_copy(out=xstage[:, :],
                                            in_=XT[:, ts(iv, BL)])
                      nc.tensor.matmul(s_ps[:, :], xstage[:, :],
                                       mov[:, ts(iv, CO)],
                                       start=False, stop=False)
                  nc.tensor.matmul(s_ps[:, :], XT[:, (NT - 1) * BL:NT * BL],
                                   mov[:, (NT - 1) * CO:NT * CO],
                                   start=False, stop=True)
                  # squash: v = s*|s| / (1+s^2)
                  nc.scalar.activation(se[:, :], s_ps[:, :],
                                       mybir.ActivationFunctionType.Copy,
                                       scale=(1.0 / R if k == 0 else 1.0))
                  nc.vector.tensor_scalar_mul(neg[:, :], se[:, :], -1.0)
                  nc.vector.tensor_max(ab[:, :], se[:, :], neg[:, :])
                  nc.vector.tensor_mul(sq[:, :], se[:, :], se[:, :])
                  nc.vector.tensor_scalar_add(den[:, :], sq[:, :], 1.0)
                  nc.vector.reciprocal(rd[:, :], den[:, :])
                  nc.vector.tensor_mul(num[:, :], se[:, :], ab[:, :])
                  nc.vector.tensor_mul(vv[:, :], num[:, :], rd[:, :])

                  if k == NITER - 1:
                      nc.sync.dma_start(out=out_d[:, :], in_=vv[:, :])
                      continue

                  nc.scalar.activation(vbf[:, :], vv[:, :],
                                       mybir.ActivationFunctionType.Copy,
                                       scale=1.0 / B)

                  # Merged G + agreement loop over the 9 r-tiles: for each of
                  # the 8 (r,i) subtiles, compute G tile (matmul over local
                  # batch), multiply by Wg into a fixed staging tile, and
                  # feed it straight into the accumulating COMP matmul that
                  # reduces i and lands results in pure-r partition layout.
                  with tc.For_i(0, RT) as iv:
                      a_ps = aps.tile([128, CO], F32, tag="a")
                      for s in range(8):
                          nc.vector.tensor_copy(
                              out=nstage[:, :],
                              in_=XN[:, ts(iv * 8 + s, 128)])
                          g_ps = gps.tile([128, CO], F32, tag="g")
                          nc.tensor.matmul(g_ps[:, :], nstage[:, :],
                                           vbf[:, :], start=True, stop=True)
                          nc.vector.tensor_tensor(
                              ptile[:, :], Wg[:, ts(iv * 8 + s, CO)],
                              g_ps[:, :], op=mybir.AluOpType.mult)
                          nc.tensor.matmul(a_ps[:, :],
                                           CMP[:, s * 128:(s + 1) * 128],
                                           ptile[:, :],
                                           start=(s == 0), stop=(s == 7))
                      a_v = a_ps[:, :].rearrange("p (o c) -> p c o", c=C)
                      nc.vector.tensor_reduce(abar[:, ts(iv, C)], a_v,
                                              axis=mybir.AxisListType.X,
                                              op=mybir.AluOpType.add)

                  # AllReduce of abar across the 8 cores
                  ar_in = dram.tile([R, C], F32, tag="arin")
                  ar_out = dram.tile([R, C], F32, tag="arout")
                  nc.sync.dma_start(
                      out=ar_in[:, :].rearrange("(t p) c -> p t c", p=128),
                      in_=abar3)
                  nc.gpsimd.collective_compute(
                      "AllReduce", mybir.AluOpType.add,
                      replica_groups=[list(range(n_cores))],
                      ins=[ar_in[:, :].opt()],
                      outs=[ar_out[:, :].opt()])
                  nc.sync.dma_start(
                      out=arr[:, :].rearrange("p (t c) -> p t c", c=C),
                      in_=ar_out[:, :].rearrange("(t p) c -> p t c", p=128))
                  nc.vector.tensor_add(b_sb[:, :], b_sb[:, :], arr[:, :])

    nc.compile()
    return nc


def _host_inputs(x, W):
    # wg[(r,i), (o,c)] -> partition-major [128, t, CO] (t = (r,i)//128)
    wg = np.ascontiguousarray(
        W.transpose(0, 3, 2, 1).reshape(R * I, CO)).astype(ml_dtypes.bfloat16)
    wg_pm = np.ascontiguousarray(
        wg.reshape(NT, 128, CO).transpose(1, 0, 2).reshape(128, NT * CO))
    m = np.arange(128)
    rep = np.zeros((128, 8 * 128), np.float32)
    cmp_ = np.zeros((128, 8 * 128), np.float32)
    for s in range(8):
        rep[:, s * 128:(s + 1) * 128] = (
            np.arange(128)[:, None] == (s * 16 + m[None, :] // 8))
        cmp_[:, s * 128:(s + 1) * 128] = (
            m[None, :] == (s * 16 + np.arange(128)[:, None] // 8))
    cmp_ = cmp_.astype(ml_dtypes.bfloat16)
    in_maps = []
    for cidx in range(NCORES):
        xc = x[cidx * BL:(cidx + 1) * BL]
        xt = np.ascontiguousarray(
            xc.transpose(1, 2, 0).reshape(R * I, BL)).astype(ml_dtypes.bfloat16)
        xt_pm = np.ascontiguousarray(
            xt.reshape(NT, 128, BL).transpose(1, 0, 2).reshape(128, NT * BL))
        xn = np.ascontiguousarray(
            xc.reshape(BL, R * I)).astype(ml_dtypes.bfloat16)
        in_maps.append({"wg": wg_pm, "xt": xt_pm, "xn": xn,
                        "rep": rep, "cmp": cmp_})
    return in_maps


def kernel(x, W):
    x = np.ascontiguousarray(np.asarray(x, dtype=np.float32))
    W = np.ascontiguousarray(np.asarray(W, dtype=np.float32))
    assert x.shape == (B, R, I) and W.shape == (R, C, O, I)
    if "nc" not in _CACHE:
        _CACHE["nc"] = _build()
    nc = _CACHE["nc"]
    in_maps = _host_inputs(x, W)
    res = bass_utils.run_bass_kernel_spmd(nc, in_maps,
                                          core_ids=list(range(NCORES)))
    vs = [r["out"].reshape(BL, O, C).transpose(0, 2, 1) for r in res.results]
    return np.concatenate(vs, axis=0)[..., None].astype(np.float32)
